# revision 5
# baseline (speedup 1.0000x reference)
"""Trainium2 Bass kernel for the stacked-Chebyshev locally-connected net.

Reference computation (B=256, k=6250, d*d=4096, O=10):
    x1 = z @ (mask*T1).T
    x2 = 2*(z @ (mask*T2).T)*x1 - T0
    x3 = 2*(z @ (mask*T3).T)*x2 - x1
    out = x3 @ C_w.T + C_b

The mask is a locally-connected conv pattern: 16x16 patch, stride 2, 25x25
positions, stacked 10x.  Rows sharing patch-row index i have a contiguous
1024-wide support in d; grouping by i cuts the contraction from 4096 to
1024.  On top of that, sorting each group's 250 k-columns by patch-column
j and splitting 125/125 narrows the support of each k-tile to 40 image
columns; with a column-interleaved z layout (partition = 2*col + row
parity, ascending for the low-j tile, descending for the high-j tile)
each k-tile contracts over only 80 of the 128 partitions of every chunk.
That cuts weight DMA traffic ~40% at zero matmul cost.

Sharding: 25 i-groups over 8 cores; every core gets 3 whole groups plus
1/8 of group 24 (a 32-column "mini" unit, j-sorted per core so its
support fits a host-shifted 48-partition window).

Performance structure:
  - every DMA is a plain 2D copy of a host-preblocked array, issued on
    the two HWDGE rings (sync, scalar) interleaved in PE consumption
    order; concurrent transfers share HBM bandwidth round-robin, so the
    critical first loads are issued first and late loads queue behind.
  - dummy matmuls on a memset tile run during the DMA preamble so the
    PE's HAM clock-gate is warm (2.4 GHz) when the real matmuls start,
    and the matmul stream is paced to never stall long enough to
    re-throttle.
  - all matmuls (layers + k->O projection) are fp16 so fast-weight-load
    stays enabled and LDWEIGHTS hides behind the matmul stream.
  - the Chebyshev recurrence is split across the scalar (ACT) and vector
    (DVE) engines; per-core partial outputs are summed on the host.
"""

import numpy as np

import concourse.bass as bass
import concourse.mybir as mybir
import concourse.tile as tile
from concourse import bacc
from concourse.bass_utils import run_bass_kernel_spmd

F32 = mybir.dt.float32
F16 = mybir.dt.float16

B = 256          # batch
O = 10           # output classes
D2 = 4096        # d*d
N_CORES = 8
FULL_SLOTS = 3   # whole groups per core
KC = 80          # contraction partitions per k-tile chunk (support window)
KM = 48          # contraction partitions for the mini unit
MINI_COLS = 32   # k-columns of the shared group-24 mini unit (<=32 real)
WIN_CH = 10      # z.T window chunks per core (slot s uses chunks s..s+7)
G_SPLIT = 24     # the group split across all 8 cores
N_UNITS = 2 * FULL_SLOTS + 1
DUMMY_MMS = 20   # HAM warm-up matmuls during the DMA preamble

# columns of group G_SPLIT owned by each core (6x31 + 2x32 = 250)
_MINI_N = (31, 31, 31, 31, 31, 31, 32, 32)
_MINI_OFF = tuple(int(x) for x in np.cumsum((0,) + _MINI_N[:-1]))


def _group_cols_jmajor(i):
    """k-column indices of group i, j-major (j outer, stack inner)."""
    return np.array(
        [s * 625 + i * 25 + j for j in range(25) for s in range(10)], dtype=np.int64
    )


# window pixel indices within a 128-px chunk (2 image rows of 64):
# ascending layout partition p <-> pixel (p%2)*64 + p//2   (cols 0..39)
# descending layout partition p <-> pixel (p%2)*64 + 63-p//2 (cols 63..24)
_WIN_A = np.array([(p % 2) * 64 + p // 2 for p in range(KC)], dtype=np.int64)
_WIN_D = np.array([(p % 2) * 64 + 63 - p // 2 for p in range(KC)], dtype=np.int64)


def _build_nc():
    nc = bacc.Bacc(
        "TRN2", target_bir_lowering=False, debug=False, num_devices=N_CORES
    )
    # z.T window in the two column-interleaved layouts, chunk-blocked
    zwa = nc.dram_tensor("zwa", [KC, WIN_CH * B], F16, kind="ExternalInput").ap()
    zwd = nc.dram_tensor("zwd", [KC, WIN_CH * B], F16, kind="ExternalInput").ap()
    # group-24 z.T slab, per-core col-shifted, chunk-blocked
    zg = nc.dram_tensor("zg", [KM, 8 * B], F16, kind="ExternalInput").ap()
    # full-slot weights; row block (s*3+li)*2 + tile is one [KC, 8*128] image
    wall = nc.dram_tensor("wall", [18 * KC, 8 * 128], F16,
                          kind="ExternalInput").ap()
    # mini-unit weights: col = layer*256 + chunk*32 + n
    wm = nc.dram_tensor("wm", [KM, 3 * 8 * MINI_COLS], F16,
                        kind="ExternalInput").ap()
    # negated T0 (additive bias on the scalar engine); col = unit index
    t0n = nc.dram_tensor("t0n", [128, 8], F32, kind="ExternalInput").ap()
    cwt = nc.dram_tensor("cwt", [128, N_UNITS * O], F16, kind="ExternalInput").ap()
    out = nc.dram_tensor("out", [O, B], F32, kind="ExternalOutput").ap()

    with tile.TileContext(nc) as tc:
        with (
            tc.tile_pool(name="zpool", bufs=1) as zpool,
            tc.tile_pool(name="cpool", bufs=1) as cpool,
            tc.tile_pool(name="wpool", bufs=1) as wpool,
            tc.tile_pool(name="xpool", bufs=6) as xpool,
            tc.tile_pool(name="tpool", bufs=4) as tpool,
            tc.tile_pool(name="ppool", bufs=6, space="PSUM") as ppool,
            tc.tile_pool(name="opool", bufs=1, space="PSUM") as opool,
            tc.tile_pool(name="dpool", bufs=1, space="PSUM") as dpool,
        ):
            # ---- dummy warm-up data (no DMA dependency) ----
            dz = zpool.tile([128, B], F16, tag="dz")
            nc.vector.memset(dz[:], 0.0)

            # ---- DMA issue plan (consumption-ordered, ring-interleaved) ----
            za1 = zpool.tile([KC, 4 * B], F16, tag="za1")
            za2 = zpool.tile([KC, 6 * B], F16, tag="za2")
            zd1 = zpool.tile([KC, 4 * B], F16, tag="zd1")
            zd2 = zpool.tile([KC, 6 * B], F16, tag="zd2")

            wt = {}   # (li, s, t) -> [KC, 1024] tile

            def wdma(eng, s, li, t):
                tl = wpool.tile([KC, 8 * 128], F16, tag=f"w{s}{li}{t}")
                wt[(li, s, t)] = tl
                r = ((3 * s + li) * 2 + t) * KC
                eng.dma_start(tl[:], wall[r:r + KC, :])

            zgt = cpool.tile([KM, 8 * B], F16, tag="zg")
            wm_sb = cpool.tile([KM, 3 * 8 * MINI_COLS], F16, tag="wm")
            t0_sb = cpool.tile([128, 8], F32, tag="t0")
            cw_sb = cpool.tile([128, N_UNITS * O], F16, tag="cw")

            # ring A (sync)
            nc.sync.dma_start(za1[:], zwa[:, 0:4 * B])
            wdma(nc.sync, 0, 0, 0)
            # ring B (scalar)
            nc.scalar.dma_start(zd1[:], zwd[:, 0:4 * B])
            wdma(nc.scalar, 0, 0, 1)
            nc.scalar.dma_start(za2[:], zwa[:, 4 * B:WIN_CH * B])
            nc.scalar.dma_start(zd2[:], zwd[:, 4 * B:WIN_CH * B])
            # remaining weights, consumption-ordered, alternating rings
            for s, li in ((0, 1), (0, 2), (1, 0), (1, 1), (1, 2),
                          (2, 0), (2, 1), (2, 2)):
                wdma(nc.sync, s, li, (s + li) % 2)
                wdma(nc.scalar, s, li, (s + li + 1) % 2)
            nc.sync.dma_start(zgt[:], zg[:])
            nc.scalar.dma_start(wm_sb[:], wm[:])
            nc.gpsimd.dma_start(t0_sb[:], t0n[:])
            nc.gpsimd.dma_start(cw_sb[:], cwt[:])

            # ---- HAM warm-up ----
            dps = dpool.tile([128, B], F32)
            for _ in range(DUMMY_MMS):
                nc.tensor.matmul(dps[:], dz[:, 0:128], dz[:], start=True,
                                 stop=True)

            def zch(t, c):
                if t == 0:
                    return za1[:, c * B:(c + 1) * B] if c < 4 else \
                        za2[:, (c - 4) * B:(c - 3) * B]
                return zd1[:, c * B:(c + 1) * B] if c < 4 else \
                    zd2[:, (c - 4) * B:(c - 3) * B]

            psum_o = opool.tile([O, B], F32)
            n_proj = 0
            pending = []   # deferred projection matmuls (src, unit, rows)

            def flush_proj():
                nonlocal n_proj
                for src_t, unit, rows in pending:
                    n_proj += 1
                    nc.tensor.matmul(psum_o[:],
                                     cw_sb[0:rows, unit * O:(unit + 1) * O],
                                     src_t[:],
                                     start=(n_proj == 1),
                                     stop=(n_proj == N_UNITS))
                pending.clear()

            def epilogue(li, p, xs, unit, rows):
                """Per-layer recurrence for one k-tile unit on ACT + DVE."""
                if li == 0:
                    x1 = xpool.tile([rows, B], F32, tag="x1")
                    nc.scalar.copy(x1[:], p[:])
                    xs["x1"] = x1
                elif li == 1:
                    m2 = tpool.tile([rows, B], F32, tag="m2")
                    x2 = xpool.tile([rows, B], F32, tag="x2")
                    nc.vector.tensor_mul(m2[:], p[:], xs["x1"][:])
                    nc.scalar.add(x2[:], m2[:], t0_sb[0:rows, unit:unit + 1])
                    xs["x2"] = x2
                else:
                    u = tpool.tile([rows, B], F32, tag="u")
                    x3 = xpool.tile([rows, B], F16, tag="x3")
                    nc.vector.tensor_mul(u[:], p[:], xs["x2"][:])
                    nc.vector.tensor_sub(x3[:], u[:], xs["x1"][:])
                    pending.append((x3, unit, rows))

            def full_slot(s):
                units = [{}, {}]
                for li in range(3):
                    flush_proj()
                    for t in range(2):
                        w = wt[(li, s, t)]
                        p = ppool.tile([128, B], F32, tag="ps")
                        for kc in range(8):
                            nc.tensor.matmul(p[:], w[:, kc * 128:(kc + 1) * 128],
                                             zch(t, s + kc),
                                             start=(kc == 0), stop=(kc == 7))
                        epilogue(li, p, units[t], 2 * s + t, 128)

            full_slot(0)
            full_slot(1)
            full_slot(2)

            # mini unit last: its short [32, B] recurrence minimizes the tail
            mini = {}
            for li in range(3):
                p = ppool.tile([MINI_COLS, B], F32, tag="ps")
                flush_proj()
                for kc in range(8):
                    lhsT = wm_sb[:, li * 8 * MINI_COLS + kc * MINI_COLS:
                                 li * 8 * MINI_COLS + (kc + 1) * MINI_COLS]
                    nc.tensor.matmul(p[:], lhsT, zgt[:, kc * B:(kc + 1) * B],
                                     start=(kc == 0), stop=(kc == 7))
                epilogue(li, p, mini, 6, MINI_COLS)
            flush_proj()

            out_sb = cpool.tile([O, B], F32, tag="out")
            nc.vector.tensor_copy(out_sb[:], psum_o[:])
            nc.sync.dma_start(out[:], out_sb[:])

    nc.compile()
    return nc


_NC = None


def _get_nc():
    global _NC
    if _NC is None:
        _NC = _build_nc()
    return _NC


def _prepare_in_maps(z, T1, T2, T3, T0, C_w, mask):
    z = np.ascontiguousarray(np.asarray(z, dtype=np.float32).reshape(B, D2))
    T1 = np.asarray(T1, dtype=np.float32)
    T2 = np.asarray(T2, dtype=np.float32)
    T3 = np.asarray(T3, dtype=np.float32)
    T0 = np.asarray(T0, dtype=np.float32)
    C_w = np.asarray(C_w, dtype=np.float32)
    mask = np.asarray(mask, dtype=np.float32)

    zT = np.ascontiguousarray(z.T)                   # [4096, 256]
    Ts = (T1, T2, T3)
    scales = (1.0, 2.0, 2.0)
    g24_cols = _group_cols_jmajor(G_SPLIT)
    g24_win = np.arange(128 * G_SPLIT, 128 * G_SPLIT + 1024)

    in_maps = []
    for c in range(N_CORES):
        i0 = 3 * c
        # z window chunks [WIN_CH, 128, B] then pick window pixel rows
        zwin = zT[128 * i0:128 * i0 + WIN_CH * 128].reshape(WIN_CH, 128, B)
        zwa_blk = (zwin[:, _WIN_A, :].transpose(1, 0, 2)
                   .reshape(KC, WIN_CH * B)).astype(np.float16)
        zwd_blk = (zwin[:, _WIN_D, :].transpose(1, 0, 2)
                   .reshape(KC, WIN_CH * B)).astype(np.float16)
        m = {"zwa": np.ascontiguousarray(zwa_blk),
             "zwd": np.ascontiguousarray(zwd_blk)}

        # full-slot weights: [18*KC, 8*128], row block (3s+li)*2+t
        wts = np.zeros((FULL_SLOTS, 3, 2, KC, 8, 128), np.float32)
        for s in range(FULL_SLOTS):
            g = i0 + s
            cols = _group_cols_jmajor(g)
            colsets = (cols[0:125], cols[125:250])
            for li, (T, sc) in enumerate(zip(Ts, scales)):
                for t, cset in enumerate(colsets):
                    ix = np.ix_(cset, np.arange(128 * g, 128 * g + 1024))
                    AT = (sc * T[ix] * mask[ix]).T      # [1024, 125]
                    AT = AT.reshape(8, 128, 125)
                    win = _WIN_A if t == 0 else _WIN_D
                    # [8, KC, 125] -> [KC, 8, 128(pad)]
                    wts[s, li, t, :, :, 0:125] = AT[:, win, :].transpose(1, 0, 2)
        m["wall"] = np.ascontiguousarray(
            wts.reshape(18 * KC, 8 * 128)).astype(np.float16)

        # mini unit: j-sorted slice of group 24, host-shifted 48-px window
        nmini = _MINI_N[c]
        o = _MINI_OFF[c]
        mcols = g24_cols[o:o + nmini]
        jmin = int(o // 10)
        # col window [2*jmin, 2*jmin+24), clipped at 64
        ccols = 2 * jmin + np.arange(KM // 2)
        valid = ccols < 64
        ccols = np.where(valid, ccols, 0)
        # partition p <-> pixel (p%2)*64 + ccols[p//2]
        win_m = np.array([(p % 2) * 64 + ccols[p // 2] for p in range(KM)],
                         dtype=np.int64)
        vmask = np.array([valid[p // 2] for p in range(KM)], np.float32)

        zgwin = zT[128 * G_SPLIT:128 * G_SPLIT + 1024].reshape(8, 128, B)
        zg_blk = (zgwin[:, win_m, :] * vmask[None, :, None]).transpose(1, 0, 2)
        m["zg"] = np.ascontiguousarray(
            zg_blk.reshape(KM, 8 * B)).astype(np.float16)

        wmh = np.zeros((KM, 3, 8, MINI_COLS), np.float32)
        for li, (T, sc) in enumerate(zip(Ts, scales)):
            A = (sc * T[np.ix_(mcols, g24_win)] * mask[np.ix_(mcols, g24_win)]).T
            A = A.reshape(8, 128, nmini)[:, win_m, :]        # [8, KM, nmini]
            wmh[:, li, :, 0:nmini] = (A * vmask[None, :, None]).transpose(1, 0, 2)
        m["wm"] = np.ascontiguousarray(
            wmh.reshape(KM, 3 * 8 * MINI_COLS)).astype(np.float16)

        # t0 / C_w per unit: units 0..5 = full slots (2s+t), 6 = mini
        t0nh = np.zeros((128, 8), np.float32)
        cwth = np.zeros((128, N_UNITS * O), np.float32)
        for s in range(FULL_SLOTS):
            cols = _group_cols_jmajor(i0 + s)
            t0nh[0:125, 2 * s] = -T0[cols[0:125]]
            t0nh[0:125, 2 * s + 1] = -T0[cols[125:250]]
            cwth[0:125, (2 * s) * O:(2 * s + 1) * O] = C_w[:, cols[0:125]].T
            cwth[0:125, (2 * s + 1) * O:(2 * s + 2) * O] = C_w[:, cols[125:250]].T
        t0nh[0:nmini, 6] = -T0[mcols]
        cwth[0:nmini, 6 * O:7 * O] = C_w[:, mcols].T
        m["t0n"] = t0nh
        m["cwt"] = cwth.astype(np.float16)
        in_maps.append(m)
    return in_maps


def kernel(z, T1, T2, T3, T0, C_w, C_b, mask):
    nc = _get_nc()
    in_maps = _prepare_in_maps(z, T1, T2, T3, T0, C_w, mask)
    res = run_bass_kernel_spmd(nc, in_maps, core_ids=list(range(N_CORES)))
    total = np.zeros((O, B), np.float32)
    for c in range(N_CORES):
        total += res.results[c]["out"]
    C_b = np.asarray(C_b, dtype=np.float32)
    return (total.T + C_b).astype(np.float32)


# revision 8
# speedup vs baseline: 1.2704x; 1.2704x over previous
"""Trainium2 Bass kernel for the stacked-Chebyshev locally-connected net.

Reference computation (B=256, k=6250, d*d=4096, O=10):
    x1 = z @ (mask*T1).T
    x2 = 2*(z @ (mask*T2).T)*x1 - T0
    x3 = 2*(z @ (mask*T3).T)*x2 - x1
    out = x3 @ C_w.T + C_b

The mask is a locally-connected conv pattern: 16x16 patch, stride 2, 25x25
positions, stacked 10x.  Rows sharing patch-row index i have a contiguous
1024-wide support in d; grouping by i cuts the contraction from 4096 to
1024.  On top of that, sorting each group's 250 k-columns by patch-column
j and splitting 125/125 narrows the support of each k-tile to 40 image
columns; with a column-interleaved z layout (partition = 2*col + row
parity, ascending for the low-j tile, descending for the high-j tile)
each k-tile contracts over only 80 of the 128 partitions of every chunk.
That cuts weight DMA traffic ~40% at zero matmul cost.

Sharding: 25 i-groups over 8 cores; every core gets 3 whole groups plus
1/8 of group 24 (a 32-column "mini" unit, j-sorted per core so its
support fits a host-shifted 48-partition window).

Performance structure:
  - every DMA is a plain 2D copy of a host-preblocked array, issued on
    the two HWDGE rings (sync, scalar) interleaved in PE consumption
    order; concurrent transfers share HBM bandwidth round-robin, so the
    critical first loads are issued first and late loads queue behind.
  - dummy matmuls on a memset tile run during the DMA preamble so the
    PE's HAM clock-gate is warm (2.4 GHz) when the real matmuls start,
    and the matmul stream is paced to never stall long enough to
    re-throttle.
  - all matmuls (layers + k->O projection) are fp16 so fast-weight-load
    stays enabled and LDWEIGHTS hides behind the matmul stream.
  - the Chebyshev recurrence is split across the scalar (ACT) and vector
    (DVE) engines; per-core partial outputs are summed on the host.
"""

import numpy as np

import concourse.bass as bass
import concourse.mybir as mybir
import concourse.tile as tile
from concourse import bacc
from concourse.bass_utils import run_bass_kernel_spmd

F32 = mybir.dt.float32
F16 = mybir.dt.float16

B = 256          # batch
O = 10           # output classes
D2 = 4096        # d*d
N_CORES = 8
FULL_SLOTS = 3   # whole groups per core
KC = 80          # contraction partitions per k-tile chunk (support window)
KM = 48          # contraction partitions for the mini unit
MINI_COLS = 32   # k-columns of the shared group-24 mini unit (<=32 real)
WIN_CH = 10      # z.T window chunks per core (slot s uses chunks s..s+7)
G_SPLIT = 24     # the group split across all 8 cores
N_UNITS = 2 * FULL_SLOTS + 1
DUMMY_MMS = 20   # HAM warm-up matmuls during the DMA preamble

# columns of group G_SPLIT owned by each core (6x31 + 2x32 = 250)
_MINI_N = (31, 31, 31, 31, 31, 31, 32, 32)
_MINI_OFF = tuple(int(x) for x in np.cumsum((0,) + _MINI_N[:-1]))


def _group_cols_jmajor(i):
    """k-column indices of group i, j-major (j outer, stack inner)."""
    return np.array(
        [s * 625 + i * 25 + j for j in range(25) for s in range(10)], dtype=np.int64
    )


# window pixel indices within a 128-px chunk (2 image rows of 64):
# ascending layout partition p <-> pixel (p%2)*64 + p//2   (cols 0..39)
# descending layout partition p <-> pixel (p%2)*64 + 63-p//2 (cols 63..24)
_WIN_A = np.array([(p % 2) * 64 + p // 2 for p in range(KC)], dtype=np.int64)
_WIN_D = np.array([(p % 2) * 64 + 63 - p // 2 for p in range(KC)], dtype=np.int64)


def _build_nc():
    nc = bacc.Bacc(
        "TRN2", target_bir_lowering=False, debug=False, num_devices=N_CORES
    )
    # z.T window in the two column-interleaved layouts, chunk-blocked
    zwa = nc.dram_tensor("zwa", [128, WIN_CH * B], F16, kind="ExternalInput").ap()
    zwd = nc.dram_tensor("zwd", [128, WIN_CH * B], F16, kind="ExternalInput").ap()
    # group-24 z.T slab, per-core col-shifted, chunk-blocked
    zg = nc.dram_tensor("zg", [128, 8 * B], F16, kind="ExternalInput").ap()
    # full-slot weights; row block (s*3+li)*2 + tile is one [KC, 8*128] image
    wall = nc.dram_tensor("wall", [18 * KC, 8 * 128], F16,
                          kind="ExternalInput").ap()
    # mini-unit weights: col = layer*256 + chunk*32 + n
    wm = nc.dram_tensor("wm", [KM, 3 * 8 * MINI_COLS], F16,
                        kind="ExternalInput").ap()
    # negated T0 (additive bias on the scalar engine); col = unit index
    t0n = nc.dram_tensor("t0n", [128, 8], F32, kind="ExternalInput").ap()
    cwt = nc.dram_tensor("cwt", [128, N_UNITS * O], F16, kind="ExternalInput").ap()
    out = nc.dram_tensor("out", [O, B], F32, kind="ExternalOutput").ap()

    with tile.TileContext(nc) as tc:
        with (
            tc.tile_pool(name="zpool", bufs=1) as zpool,
            tc.tile_pool(name="cpool", bufs=1) as cpool,
            tc.tile_pool(name="wpool", bufs=1) as wpool,
            tc.tile_pool(name="xpool", bufs=6) as xpool,
            tc.tile_pool(name="tpool", bufs=4) as tpool,
            tc.tile_pool(name="ppool", bufs=6, space="PSUM") as ppool,
            tc.tile_pool(name="opool", bufs=1, space="PSUM") as opool,
            tc.tile_pool(name="dpool", bufs=1, space="PSUM") as dpool,
        ):
            # ---- dummy warm-up data (no DMA dependency) ----
            dz = zpool.tile([128, B], F16, tag="dz")
            nc.vector.memset(dz[:], 0.0)

            # ---- DMA issue plan (consumption-ordered, ring-interleaved) ----
            za1 = zpool.tile([128, 4 * B], F16, tag="za1")
            za2 = zpool.tile([128, 6 * B], F16, tag="za2")
            zd1 = zpool.tile([128, 4 * B], F16, tag="zd1")
            zd2 = zpool.tile([128, 6 * B], F16, tag="zd2")

            wt = {}   # (li, s, t) -> [KC, 1024] tile

            def wdma(eng, s, li, t):
                tl = wpool.tile([128, 8 * 128], F16, tag=f"w{s}{li}{t}")
                wt[(li, s, t)] = tl
                r = ((3 * s + li) * 2 + t) * KC
                eng.dma_start(tl[0:KC, :], wall[r:r + KC, :])

            zgt = cpool.tile([128, 8 * B], F16, tag="zg")
            wm_sb = cpool.tile([128, 3 * 8 * MINI_COLS], F16, tag="wm")
            t0_sb = cpool.tile([128, 8], F32, tag="t0")
            cw_sb = cpool.tile([128, N_UNITS * O], F16, tag="cw")

            # ring A (sync)
            nc.sync.dma_start(za1[:], zwa[:, 0:4 * B])
            wdma(nc.sync, 0, 0, 0)
            # ring B (scalar)
            nc.scalar.dma_start(zd1[:], zwd[:, 0:4 * B])
            wdma(nc.scalar, 0, 0, 1)
            nc.scalar.dma_start(za2[:], zwa[:, 4 * B:WIN_CH * B])
            nc.sync.dma_start(zd2[:], zwd[:, 4 * B:WIN_CH * B])
            # remaining weights, consumption-ordered, alternating rings
            for s, li in ((0, 1), (0, 2), (1, 0), (1, 1), (1, 2),
                          (2, 0), (2, 1), (2, 2)):
                wdma(nc.sync, s, li, (s + li) % 2)
                wdma(nc.scalar, s, li, (s + li + 1) % 2)
            nc.sync.dma_start(zgt[:], zg[:])
            nc.scalar.dma_start(wm_sb[0:KM, :], wm[:])
            nc.gpsimd.dma_start(t0_sb[:], t0n[:])
            nc.gpsimd.dma_start(cw_sb[:], cwt[:])

            # ---- HAM warm-up ----
            dps = dpool.tile([128, B], F32)
            for _ in range(DUMMY_MMS):
                nc.tensor.matmul(dps[:], dz[:, 0:128], dz[:], start=True,
                                 stop=True)

            def zch(t, c):
                if t == 0:
                    return za1[:, c * B:(c + 1) * B] if c < 4 else \
                        za2[:, (c - 4) * B:(c - 3) * B]
                return zd1[:, c * B:(c + 1) * B] if c < 4 else \
                    zd2[:, (c - 4) * B:(c - 3) * B]

            psum_o = opool.tile([O, B], F32)
            n_proj = 0
            pending = []   # deferred projection matmuls (src, unit, rows)

            def flush_proj():
                nonlocal n_proj
                for src_t, unit, rows in pending:
                    n_proj += 1
                    nc.tensor.matmul(psum_o[:],
                                     cw_sb[0:rows, unit * O:(unit + 1) * O],
                                     src_t[:],
                                     start=(n_proj == 1),
                                     stop=(n_proj == N_UNITS))
                pending.clear()

            def epilogue(li, p, xs, unit, rows):
                """Per-layer recurrence for one k-tile unit on ACT + DVE."""
                if li == 0:
                    x1 = xpool.tile([rows, B], F32, tag="x1")
                    nc.scalar.copy(x1[:], p[:])
                    xs["x1"] = x1
                elif li == 1:
                    m2 = tpool.tile([rows, B], F32, tag="m2")
                    x2 = xpool.tile([rows, B], F32, tag="x2")
                    nc.vector.tensor_mul(m2[:], p[:], xs["x1"][:])
                    nc.scalar.add(x2[:], m2[:], t0_sb[0:rows, unit:unit + 1])
                    xs["x2"] = x2
                else:
                    u = tpool.tile([rows, B], F32, tag="u")
                    x3 = xpool.tile([rows, B], F16, tag="x3")
                    nc.vector.tensor_mul(u[:], p[:], xs["x2"][:])
                    nc.vector.tensor_sub(x3[:], u[:], xs["x1"][:])
                    pending.append((x3, unit, rows))

            def full_slot(s):
                units = [{}, {}]
                for li in range(3):
                    flush_proj()
                    for t in range(2):
                        w = wt[(li, s, t)]
                        p = ppool.tile([128, B], F32, tag="ps")
                        for kc in range(8):
                            nc.tensor.matmul(p[:], w[:, kc * 128:(kc + 1) * 128],
                                             zch(t, s + kc),
                                             start=(kc == 0), stop=(kc == 7))
                        epilogue(li, p, units[t], 2 * s + t, 128)

            full_slot(0)
            full_slot(1)
            full_slot(2)

            # mini unit last: its short [32, B] recurrence minimizes the tail
            mini = {}
            for li in range(3):
                p = ppool.tile([MINI_COLS, B], F32, tag="ps")
                flush_proj()
                for kc in range(8):
                    lhsT = wm_sb[:, li * 8 * MINI_COLS + kc * MINI_COLS:
                                 li * 8 * MINI_COLS + (kc + 1) * MINI_COLS]
                    nc.tensor.matmul(p[:], lhsT, zgt[:, kc * B:(kc + 1) * B],
                                     start=(kc == 0), stop=(kc == 7))
                epilogue(li, p, mini, 6, MINI_COLS)
            flush_proj()

            out_sb = cpool.tile([O, B], F32, tag="out")
            nc.vector.tensor_copy(out_sb[:], psum_o[:])
            nc.sync.dma_start(out[:], out_sb[:])

    nc.compile()
    return nc


_NC = None


def _get_nc():
    global _NC
    if _NC is None:
        _NC = _build_nc()
    return _NC


def _prepare_in_maps(z, T1, T2, T3, T0, C_w, mask):
    z = np.ascontiguousarray(np.asarray(z, dtype=np.float32).reshape(B, D2))
    T1 = np.asarray(T1, dtype=np.float32)
    T2 = np.asarray(T2, dtype=np.float32)
    T3 = np.asarray(T3, dtype=np.float32)
    T0 = np.asarray(T0, dtype=np.float32)
    C_w = np.asarray(C_w, dtype=np.float32)
    mask = np.asarray(mask, dtype=np.float32)

    zT = np.ascontiguousarray(z.T)                   # [4096, 256]
    Ts = (T1, T2, T3)
    scales = (1.0, 2.0, 2.0)
    g24_cols = _group_cols_jmajor(G_SPLIT)
    g24_win = np.arange(128 * G_SPLIT, 128 * G_SPLIT + 1024)

    in_maps = []
    for c in range(N_CORES):
        i0 = 3 * c
        # z window chunks [WIN_CH, 128, B] then pick window pixel rows
        zwin = zT[128 * i0:128 * i0 + WIN_CH * 128].reshape(WIN_CH, 128, B)
        zwa_blk = np.zeros((128, WIN_CH * B), np.float16)
        zwd_blk = np.zeros((128, WIN_CH * B), np.float16)
        zwa_blk[0:KC] = (zwin[:, _WIN_A, :].transpose(1, 0, 2)
                         .reshape(KC, WIN_CH * B)).astype(np.float16)
        zwd_blk[0:KC] = (zwin[:, _WIN_D, :].transpose(1, 0, 2)
                         .reshape(KC, WIN_CH * B)).astype(np.float16)
        m = {"zwa": zwa_blk, "zwd": zwd_blk}

        # full-slot weights: [18*KC, 8*128], row block (3s+li)*2+t
        wts = np.zeros((FULL_SLOTS, 3, 2, KC, 8, 128), np.float32)
        for s in range(FULL_SLOTS):
            g = i0 + s
            cols = _group_cols_jmajor(g)
            colsets = (cols[0:125], cols[125:250])
            for li, (T, sc) in enumerate(zip(Ts, scales)):
                for t, cset in enumerate(colsets):
                    ix = np.ix_(cset, np.arange(128 * g, 128 * g + 1024))
                    AT = (sc * T[ix] * mask[ix]).T      # [1024, 125]
                    AT = AT.reshape(8, 128, 125)
                    win = _WIN_A if t == 0 else _WIN_D
                    # [8, KC, 125] -> [KC, 8, 128(pad)]
                    wts[s, li, t, :, :, 0:125] = AT[:, win, :].transpose(1, 0, 2)
        m["wall"] = np.ascontiguousarray(
            wts.reshape(18 * KC, 8 * 128)).astype(np.float16)

        # mini unit: j-sorted slice of group 24, host-shifted 48-px window
        nmini = _MINI_N[c]
        o = _MINI_OFF[c]
        mcols = g24_cols[o:o + nmini]
        jmin = int(o // 10)
        # col window [2*jmin, 2*jmin+24), clipped at 64
        ccols = 2 * jmin + np.arange(KM // 2)
        valid = ccols < 64
        ccols = np.where(valid, ccols, 0)
        # partition p <-> pixel (p%2)*64 + ccols[p//2]
        win_m = np.array([(p % 2) * 64 + ccols[p // 2] for p in range(KM)],
                         dtype=np.int64)
        vmask = np.array([valid[p // 2] for p in range(KM)], np.float32)

        zgwin = zT[128 * G_SPLIT:128 * G_SPLIT + 1024].reshape(8, 128, B)
        zg_blk = (zgwin[:, win_m, :] * vmask[None, :, None]).transpose(1, 0, 2)
        zg_full = np.zeros((128, 8 * B), np.float16)
        zg_full[0:KM] = zg_blk.reshape(KM, 8 * B).astype(np.float16)
        m["zg"] = zg_full

        wmh = np.zeros((KM, 3, 8, MINI_COLS), np.float32)
        for li, (T, sc) in enumerate(zip(Ts, scales)):
            A = (sc * T[np.ix_(mcols, g24_win)] * mask[np.ix_(mcols, g24_win)]).T
            A = A.reshape(8, 128, nmini)[:, win_m, :]        # [8, KM, nmini]
            wmh[:, li, :, 0:nmini] = (A * vmask[None, :, None]).transpose(1, 0, 2)
        m["wm"] = np.ascontiguousarray(
            wmh.reshape(KM, 3 * 8 * MINI_COLS)).astype(np.float16)

        # t0 / C_w per unit: units 0..5 = full slots (2s+t), 6 = mini
        t0nh = np.zeros((128, 8), np.float32)
        cwth = np.zeros((128, N_UNITS * O), np.float32)
        for s in range(FULL_SLOTS):
            cols = _group_cols_jmajor(i0 + s)
            t0nh[0:125, 2 * s] = -T0[cols[0:125]]
            t0nh[0:125, 2 * s + 1] = -T0[cols[125:250]]
            cwth[0:125, (2 * s) * O:(2 * s + 1) * O] = C_w[:, cols[0:125]].T
            cwth[0:125, (2 * s + 1) * O:(2 * s + 2) * O] = C_w[:, cols[125:250]].T
        t0nh[0:nmini, 6] = -T0[mcols]
        cwth[0:nmini, 6 * O:7 * O] = C_w[:, mcols].T
        m["t0n"] = t0nh
        m["cwt"] = cwth.astype(np.float16)
        in_maps.append(m)
    return in_maps


def kernel(z, T1, T2, T3, T0, C_w, C_b, mask):
    nc = _get_nc()
    in_maps = _prepare_in_maps(z, T1, T2, T3, T0, C_w, mask)
    res = run_bass_kernel_spmd(nc, in_maps, core_ids=list(range(N_CORES)))
    total = np.zeros((O, B), np.float32)
    for c in range(N_CORES):
        total += res.results[c]["out"]
    C_b = np.asarray(C_b, dtype=np.float32)
    return (total.T + C_b).astype(np.float32)
